# revision 9
# baseline (speedup 1.0000x reference)
"""ContextQueryAttention (BiDAF-style) Trainium2 kernel, 8-core data-parallel.

Math (per batch):
  s[i,j]  = wq.q_j + wc.c_i + sum_d c_id * wcq_d * q_jd          (L1 x L2)
  s1      = softmax_i(s * mq_j + (1-mq_j)*NEG)                   (softmax over i)
  s2      = softmax_i(s * mp_i + (1-mp_i)*NEG)
  a       = s1 @ Q                 (L1 x D)
  b       = (s1 @ s2^T) @ C  ==  s1 @ (s2^T @ C)   <- reassociated, no L1xL1
  out     = [C, a, C*a, C*b]                                      (L1 x 4D)

Key kernel facts:
 - scores ~ N(0,1): no max-subtraction needed for a stable softmax.
 - E1 path, ST layout [j part, i free]: E1 = exp(mq_j*(s+1000) - 1000*mq_j);
   mq_j per-partition -> fused into the ACT exp (masked col -> exp(0)=1 ->
   uniform 1/L1, exactly matching the reference). Z1 via ACT accum_out.
 - E2 path, natural layout [i part, j free] from a 2nd score matmul:
   E2 = exp(mp_i*dot + (mp_i*(cwc_i+1000) - 1000)); the dropped qwq_j term
   is constant over unmasked i so it cancels in the i-softmax; masked
   entries sit at ~exp(-1000) = 0.  Z2 via a ones-column in the t matmul.
"""

import numpy as np

import concourse.bass as bass
import concourse.mybir as mybir
import concourse.tile as tile
from concourse import bacc
from concourse import bass_utils
from concourse.masks import make_identity

F32 = mybir.dt.float32
EXP = mybir.ActivationFunctionType.Exp
ADD = mybir.AluOpType.add
MULT = mybir.AluOpType.mult

B, L1, L2, D = 16, 2048, 512, 128
NCORES = 8
BPC = B // NCORES          # batches per core
NT1 = L1 // 128            # 16 i-tiles
NT2 = L2 // 128            # 4  j-tiles
SHIFT = 1000.0             # score shift making masked entries underflow exp


def _build_program(dbg=False):
    nc = bacc.Bacc("TRN2", target_bir_lowering=False, debug=False)

    ctx_d = nc.dram_tensor("context", [BPC, L1, D], F32, kind="ExternalInput").ap()
    qry_d = nc.dram_tensor("query", [BPC, L2, D], F32, kind="ExternalInput").ap()
    w_d = nc.dram_tensor("w", [3, D], F32, kind="ExternalInput").ap()
    mp_d = nc.dram_tensor("mask_p", [BPC, L1], F32, kind="ExternalInput").ap()
    mq_d = nc.dram_tensor("mask_q", [BPC, L2], F32, kind="ExternalInput").ap()
    out_d = nc.dram_tensor("out", [BPC, L1, 4 * D], F32, kind="ExternalOutput").ap()
    dbg_d = {}
    if dbg:
        for name, shape in {
            "dbg_e1": [128, NT2, L1],
            "dbg_e2n": [128, NT1, L2],
            "dbg_z1": [128, NT2],
            "dbg_qwq": [128, NT2],
            "dbg_cwc_row": [1, L1],
            "dbg_bias2": [128, NT1],
            "dbg_rhs_ab": [128, NT2, 256],
            "dbg_ct": [128, NT1, 128],
            "dbg_qt": [128, NT2, 128],
        }.items():
            dbg_d[name] = nc.dram_tensor(name, shape, F32, kind="ExternalOutput").ap()

    with tile.TileContext(nc) as tc:
        with (
            tc.tile_pool(name="const", bufs=1) as const,
            tc.tile_pool(name="big", bufs=1) as big,
            tc.tile_pool(name="work", bufs=2) as work,
            tc.tile_pool(name="outp", bufs=3) as outp,
            tc.tile_pool(name="ps512", bufs=2, space="PSUM") as ps512,
            tc.tile_pool(name="ps256", bufs=3, space="PSUM") as ps256,
            tc.tile_pool(name="psrow", bufs=1, space="PSUM") as psrow,
        ):
            ident = const.tile([128, 128], F32)
            make_identity(nc, ident)
            ones_row = const.tile([1, 128], F32)
            nc.vector.memset(ones_row, 1.0)
            shift_col = const.tile([128, 1], F32)
            nc.vector.memset(shift_col, SHIFT)
            w_sb = const.tile([128, 3], F32)  # cols: wq, wc, wcq
            nc.sync.dma_start(out=w_sb, in_=w_d.rearrange("k d -> d k"))

            for b in range(BPC):
                # ---- inputs for this batch ----
                c1 = big.tile([128, NT1, 129], F32, tag="c1")  # C | ones col @128
                nc.sync.dma_start(
                    out=c1[:, :, 0:128],
                    in_=ctx_d[b].rearrange("(t p) d -> p t d", p=128),
                )
                nc.vector.memset(c1[:, :, 128:129], 1.0)
                qn = big.tile([128, NT2, 128], F32, tag="qn")
                nc.sync.dma_start(
                    out=qn, in_=qry_d[b].rearrange("(t p) d -> p t d", p=128)
                )
                mp_sb = work.tile([128, NT1], F32, tag="mp")
                nc.sync.dma_start(out=mp_sb, in_=mp_d[b].rearrange("(t p) -> p t", p=128))
                mq_sb = work.tile([128, NT2], F32, tag="mq")
                nc.sync.dma_start(out=mq_sb, in_=mq_d[b].rearrange("(t p) -> p t", p=128))

                # ---- transposes: CT [d, i], QT [d, j] ----
                ct = big.tile([128, NT1, 128], F32, tag="ct")
                for it in range(NT1):
                    ps = ps256.tile([128, 128], F32, tag="acc")
                    nc.tensor.transpose(ps, c1[:, it, 0:128], ident)
                    nc.scalar.copy(ct[:, it, :], ps)
                qt = big.tile([128, NT2, 128], F32, tag="qt")
                for jt in range(NT2):
                    ps = ps256.tile([128, 128], F32, tag="acc")
                    nc.tensor.transpose(ps, qn[:, jt, :], ident)
                    nc.scalar.copy(qt[:, jt, :], ps)

                # ---- weighted transposes ----
                qtw = big.tile([128, NT2, 128], F32, tag="qtw")  # wcq_d * Q^T
                nc.vector.tensor_scalar_mul(qtw, qt, w_sb[:, 2:3])
                cwt = big.tile([128, NT1, 128], F32, tag="cwt")  # wcq_d * C^T
                nc.vector.tensor_scalar_mul(cwt, ct, w_sb[:, 2:3])

                # ---- cwc row [1, L1] (+SHIFT) and qwq col [128, NT2] ----
                cwc_row = work.tile([1, L1], F32, tag="cwc_row")
                for n in range(L1 // 512):
                    psr = psrow.tile([1, 512], F32, tag="cwcr")
                    nc.tensor.matmul(
                        psr, w_sb[:, 1:2], ct[:, 4 * n : 4 * (n + 1), :],
                        start=True, stop=True,
                    )
                    nc.scalar.activation(
                        cwc_row[:, 512 * n : 512 * (n + 1)], psr,
                        mybir.ActivationFunctionType.Identity,
                        bias=shift_col[0:1, :],
                    )
                qwq_sb = work.tile([128, NT2], F32, tag="qwq")
                for jt in range(NT2):
                    psq = ps256.tile([128, 1], F32, tag="acc")
                    nc.tensor.matmul(psq, qt[:, jt, :], w_sb[:, 0:1], start=True, stop=True)
                    nc.scalar.copy(qwq_sb[:, jt : jt + 1], psq)
                # cwc in natural (per-partition) form [i_part, it]
                cwc_nat = work.tile([128, NT1], F32, tag="cwc_nat")
                for it in range(NT1):
                    psc = ps256.tile([128, 1], F32, tag="acc")
                    nc.tensor.matmul(psc, ct[:, it, :], w_sb[:, 1:2], start=True, stop=True)
                    nc.scalar.copy(cwc_nat[:, it : it + 1], psc)

                # ---- per-partition exp coefficients ----
                bias1 = work.tile([128, NT2], F32, tag="bias1")  # -SHIFT*mq
                nc.vector.tensor_scalar_mul(bias1, mq_sb, -SHIFT)
                bias2 = work.tile([128, NT1], F32, tag="bias2")  # mp*(cwc+SHIFT)-SHIFT
                nc.vector.scalar_tensor_tensor(
                    out=bias2, in0=cwc_nat, scalar=SHIFT, in1=mp_sb,
                    op0=ADD, op1=MULT,
                )
                nc.vector.tensor_scalar_add(bias2, bias2, -SHIFT)

                # ---- E1: ST layout [j part, i free] ----
                e1 = big.tile([128, NT2, L1], F32, tag="e1")
                z1 = work.tile([128, NT2], F32, tag="z1")
                for jt in range(NT2):
                    st_sb = work.tile([128, L1], F32, tag="st_sb")
                    for n in range(L1 // 512):
                        psst = ps512.tile([128, 512], F32, tag="mm512")
                        nc.tensor.matmul(
                            psst, qtw[:, jt, :], ct[:, 4 * n : 4 * (n + 1), :],
                            start=True, stop=False,
                        )
                        nc.tensor.matmul(
                            psst, ones_row, cwc_row[:, 512 * n : 512 * (n + 1)],
                            start=False, stop=True,
                        )
                        nc.vector.tensor_scalar_add(
                            st_sb[:, 512 * n : 512 * (n + 1)], psst,
                            qwq_sb[:, jt : jt + 1],
                        )
                    nc.scalar.activation(
                        e1[:, jt, :], st_sb, EXP,
                        bias=bias1[:, jt : jt + 1], scale=mq_sb[:, jt : jt + 1],
                        accum_out=z1[:, jt : jt + 1],
                    )

                # ---- E2: natural layout [i part, j free] ----
                e2n = big.tile([128, NT1, L2], F32, tag="e2n")
                for it in range(NT1):
                    pss = ps512.tile([128, 512], F32, tag="mm512")
                    nc.tensor.matmul(pss, cwt[:, it, :], qt, start=True, stop=True)
                    nc.scalar.activation(
                        e2n[:, it, :], pss, EXP,
                        bias=bias2[:, it : it + 1], scale=mp_sb[:, it : it + 1],
                    )

                # ---- t = s2^T @ C (with Z2 via ones column), rhs_ab = [Q/Z1 | t] ----
                rz1 = work.tile([128, NT2], F32, tag="rz1")
                nc.vector.reciprocal(rz1, z1)
                rhs_ab = big.tile([128, NT2, 256], F32, tag="rhs_ab")
                for jt in range(NT2):
                    pst = ps256.tile([128, 129], F32, tag="acc")
                    for it in range(NT1):
                        nc.tensor.matmul(
                            pst, e2n[:, it, jt * 128 : (jt + 1) * 128], c1[:, it, 0:129],
                            start=(it == 0), stop=(it == NT1 - 1),
                        )
                    rz2 = work.tile([128, 1], F32, tag="rz2")
                    nc.vector.reciprocal(rz2, pst[:, 128:129])
                    rz12 = work.tile([128, 1], F32, tag="rz12")
                    nc.vector.tensor_mul(rz12, rz2, rz1[:, jt : jt + 1])
                    nc.vector.tensor_scalar_mul(rhs_ab[:, jt, 128:256], pst[:, 0:128], rz12)
                    nc.vector.tensor_scalar_mul(
                        rhs_ab[:, jt, 0:128], qn[:, jt, :], rz1[:, jt : jt + 1]
                    )

                # ---- [a | b] = E1^T @ rhs_ab ; assemble output ----
                for it in range(NT1):
                    psab = ps256.tile([128, 256], F32, tag="acc")
                    for jt in range(NT2):
                        nc.tensor.matmul(
                            psab, e1[:, jt, it * 128 : (it + 1) * 128], rhs_ab[:, jt, :],
                            start=(jt == 0), stop=(jt == NT2 - 1),
                        )
                    o_sb = outp.tile([128, 384], F32, tag="o_sb")
                    nc.scalar.copy(o_sb[:, 0:128], psab[:, 0:128])
                    nc.vector.tensor_mul(o_sb[:, 128:256], c1[:, it, 0:128], psab[:, 0:128])
                    nc.vector.tensor_mul(o_sb[:, 256:384], c1[:, it, 0:128], psab[:, 128:256])
                    nc.sync.dma_start(
                        out=out_d[b, it * 128 : (it + 1) * 128, 128:512], in_=o_sb
                    )
                    nc.sync.dma_start(
                        out=out_d[b, it * 128 : (it + 1) * 128, 0:128],
                        in_=c1[:, it, 0:128],
                    )

                if dbg and b == 0:
                    for name, src in {
                        "dbg_e1": e1, "dbg_e2n": e2n, "dbg_z1": z1,
                        "dbg_qwq": qwq_sb, "dbg_cwc_row": cwc_row,
                        "dbg_bias2": bias2, "dbg_rhs_ab": rhs_ab,
                        "dbg_ct": ct, "dbg_qt": qt,
                    }.items():
                        nc.sync.dma_start(out=dbg_d[name], in_=src)

    nc.compile()
    return nc


_NC = None


def _get_nc():
    global _NC
    if _NC is None:
        _NC = _build_program()
    return _NC


def _make_in_maps(inputs):
    context, query, w = inputs["context"], inputs["query"], inputs["w"]
    w2 = np.ascontiguousarray(np.asarray(w).reshape(3, D).astype(np.float32))
    mp = np.asarray(inputs["mask_p"]).astype(np.float32)
    mq = np.asarray(inputs["mask_q"]).astype(np.float32)
    in_maps = []
    for c in range(NCORES):
        sl = slice(c * BPC, (c + 1) * BPC)
        in_maps.append(
            {
                "context": np.ascontiguousarray(context[sl]),
                "query": np.ascontiguousarray(query[sl]),
                "w": w2,
                "mask_p": np.ascontiguousarray(mp[sl]),
                "mask_q": np.ascontiguousarray(mq[sl]),
            }
        )
    return in_maps


def kernel(context, query, w, mask_p, mask_q):
    nc = _get_nc()
    in_maps = _make_in_maps(
        {"context": context, "query": query, "w": w, "mask_p": mask_p, "mask_q": mask_q}
    )
    res = bass_utils.run_bass_kernel_spmd(nc, in_maps, core_ids=list(range(NCORES)))
    return np.concatenate([res.results[c]["out"] for c in range(NCORES)], axis=0)


# revision 13
# speedup vs baseline: 1.9673x; 1.9673x over previous
"""ContextQueryAttention (BiDAF-style) Trainium2 kernel, 8-core data-parallel.

Math (per batch):
  s[i,j]  = wq.q_j + wc.c_i + sum_d c_id * wcq_d * q_jd          (L1 x L2)
  s1      = softmax_i(s * mq_j + (1-mq_j)*NEG)                   (softmax over i)
  s2      = softmax_i(s * mp_i + (1-mp_i)*NEG)
  a       = s1 @ Q                 (L1 x D)
  b       = (s1 @ s2^T) @ C  ==  s1 @ (s2^T @ C)   <- reassociated, no L1xL1
  out     = [C, a, C*a, C*b]                                      (L1 x 4D)

Key kernel facts:
 - scores ~ N(0,1): no max-subtraction needed for a stable softmax.
 - qwq_j is constant along the softmax axis (i) in both softmaxes, so it
   cancels in s1 and s2 entirely and is never computed.
 - E1 path, ST layout [j part, i free]: E1 = exp(mq_j*(dot+cwc_i+1000) -
   1000*mq_j); cwc_i+1000 added in f32 via a DMA-broadcast row (bf16 would
   quantize +-2 at magnitude 1000); masked col -> exp(0)=1 -> uniform 1/L1,
   exactly matching the reference. Z1 via ACT accum_out.
 - E2 path, natural layout [i part, j free] from a 2nd score matmul:
   E2 = exp(mp_i*dot + (mp_i*(cwc_i+1000) - 1000)) fully fused in one ACT op
   (per-partition scale+bias); masked entries underflow to exactly 0.
   Z2 via a ones-column appended to C in the t matmul.
 - matmul operands bf16 (fp32 matmul runs as 2 HW passes + slow LDWEIGHTS),
   accumulation f32 in PSUM; exp inputs/outputs of the score pipeline stay
   f32 until the post-exp tensors (values in [e-6, e6], bf16-safe).
"""

import numpy as np

import concourse.bass as bass
import concourse.mybir as mybir
import concourse.tile as tile
from concourse import bacc
from concourse import bass_utils
from concourse.masks import make_identity

F32 = mybir.dt.float32
BF16 = mybir.dt.bfloat16
EXP = mybir.ActivationFunctionType.Exp
IDENT = mybir.ActivationFunctionType.Identity
ADD = mybir.AluOpType.add
MULT = mybir.AluOpType.mult

B, L1, L2, D = 16, 2048, 512, 128
NCORES = 8
BPC = B // NCORES          # batches per core
NT1 = L1 // 128            # 16 i-tiles
NT2 = L2 // 128            # 4  j-tiles
SHIFT = 1000.0             # makes masked E2 entries underflow exp to 0.0


def _build_program(dbg=False):
    nc = bacc.Bacc("TRN2", target_bir_lowering=False, debug=False)

    ctx_d = nc.dram_tensor("context", [BPC, L1, D], F32, kind="ExternalInput").ap()
    qry_d = nc.dram_tensor("query", [BPC, L2, D], F32, kind="ExternalInput").ap()
    w_d = nc.dram_tensor("w", [3, D], F32, kind="ExternalInput").ap()
    mp_d = nc.dram_tensor("mask_p", [BPC, L1], F32, kind="ExternalInput").ap()
    mq_d = nc.dram_tensor("mask_q", [BPC, L2], F32, kind="ExternalInput").ap()
    out_d = nc.dram_tensor("out", [BPC, L1, 4 * D], F32, kind="ExternalOutput").ap()
    dbg_srcs = {}

    with tile.TileContext(nc) as tc:
        with (
            tc.tile_pool(name="const", bufs=1) as const,
            tc.tile_pool(name="big", bufs=1) as big,
            tc.tile_pool(name="work", bufs=2) as work,
            tc.tile_pool(name="outp", bufs=3) as outp,
            tc.tile_pool(name="ps512", bufs=3, space="PSUM") as ps512,
            tc.tile_pool(name="ps256", bufs=3, space="PSUM") as ps256,
            tc.tile_pool(name="psrow", bufs=1, space="PSUM") as psrow,
        ):
            ident_b = const.tile([128, 128], BF16)
            make_identity(nc, ident_b)
            w_sb = const.tile([128, 3], F32)  # cols: wq, wc, wcq
            nc.sync.dma_start(out=w_sb, in_=w_d.rearrange("k d -> d k"))
            w_b = const.tile([128, 3], BF16)
            nc.vector.tensor_copy(w_b, w_sb)
            shift_col = const.tile([128, 1], F32)
            nc.vector.memset(shift_col, SHIFT)

            for b in range(BPC):
                # ---- inputs for this batch ----
                c1 = big.tile([128, NT1, 128], F32, tag="c1")
                nc.sync.dma_start(
                    out=c1, in_=ctx_d[b].rearrange("(t p) d -> p t d", p=128)
                )
                c1b = big.tile([128, NT1, 129], BF16, tag="c1b")  # bf16 C | ones
                nc.gpsimd.tensor_copy(c1b[:, :, 0:128], c1)
                nc.vector.memset(c1b[:, :, 128:129], 1.0)
                qn = big.tile([128, NT2, 128], F32, tag="qn")
                nc.sync.dma_start(
                    out=qn, in_=qry_d[b].rearrange("(t p) d -> p t d", p=128)
                )
                qnb = big.tile([128, NT2, 128], BF16, tag="qnb")
                nc.gpsimd.tensor_copy(qnb, qn)
                mp_sb = work.tile([128, NT1], F32, tag="mp")
                nc.sync.dma_start(out=mp_sb, in_=mp_d[b].rearrange("(t p) -> p t", p=128))
                mq_sb = work.tile([128, NT2], F32, tag="mq")
                nc.sync.dma_start(out=mq_sb, in_=mq_d[b].rearrange("(t p) -> p t", p=128))

                # ---- transposes: CT [d, i], QT [d, j] (bf16) ----
                ct = big.tile([128, NT1, 128], BF16, tag="ct")
                for it in range(NT1):
                    ps = ps256.tile([128, 128], BF16, tag="acc")
                    nc.tensor.transpose(ps, c1b[:, it, 0:128], ident_b)
                    nc.scalar.copy(ct[:, it, :], ps)
                qt = big.tile([128, NT2, 128], BF16, tag="qt")
                for jt in range(NT2):
                    ps = ps256.tile([128, 128], BF16, tag="acc")
                    nc.tensor.transpose(ps, qnb[:, jt, :], ident_b)
                    nc.scalar.copy(qt[:, jt, :], ps)

                # ---- wcq-weighted transposes ----
                qtw = big.tile([128, NT2, 128], BF16, tag="qtw")
                nc.vector.tensor_scalar_mul(qtw, qt, w_sb[:, 2:3])
                cwt = big.tile([128, NT1, 128], BF16, tag="cwt")
                nc.vector.tensor_scalar_mul(cwt, ct, w_sb[:, 2:3])

                # ---- cwc row [1, L1] (+SHIFT), f32 broadcast tile ----
                cwc_row = work.tile([1, L1], F32, tag="cwc_row")
                for n in range(L1 // 512):
                    psr = psrow.tile([1, 512], F32, tag="cwcr")
                    nc.tensor.matmul(
                        psr, w_b[:, 1:2], ct[:, 4 * n : 4 * (n + 1), :],
                        start=True, stop=True,
                    )
                    nc.scalar.activation(
                        cwc_row[:, 512 * n : 512 * (n + 1)], psr, IDENT,
                        bias=shift_col[0:1, :],
                    )
                cwc_bc = big.tile([128, L1], F32, tag="cwc_bc")
                nc.gpsimd.partition_broadcast(cwc_bc, cwc_row)

                # cwc in natural (per-partition) form [i_part, it]
                cwc_nat = work.tile([128, NT1], F32, tag="cwc_nat")
                for it in range(NT1):
                    psc = ps256.tile([128, 1], F32, tag="acc")
                    nc.tensor.matmul(psc, ct[:, it, :], w_b[:, 1:2], start=True, stop=True)
                    nc.scalar.copy(cwc_nat[:, it : it + 1], psc)

                # ---- per-partition exp coefficients ----
                bias1 = work.tile([128, NT2], F32, tag="bias1")  # -SHIFT*mq
                nc.vector.tensor_scalar_mul(bias1, mq_sb, -SHIFT)
                bias2 = work.tile([128, NT1], F32, tag="bias2")  # mp*(cwc+SHIFT)-SHIFT
                nc.vector.scalar_tensor_tensor(
                    out=bias2, in0=cwc_nat, scalar=SHIFT, in1=mp_sb,
                    op0=ADD, op1=MULT,
                )
                nc.vector.tensor_scalar_add(bias2, bias2, -SHIFT)

                # ---- E1: ST layout [j part, i free] ----
                e1 = big.tile([128, NT2, L1], BF16, tag="e1")
                z1 = work.tile([128, NT2], F32, tag="z1")
                for jt in range(NT2):
                    st_sb = work.tile([128, L1], F32, tag="st_sb")
                    for n in range(L1 // 512):
                        psst = ps512.tile([128, 512], F32, tag="mm512")
                        nc.tensor.matmul(
                            psst, qtw[:, jt, :], ct[:, 4 * n : 4 * (n + 1), :],
                            start=True, stop=True,
                        )
                        nc.vector.tensor_tensor(
                            st_sb[:, 512 * n : 512 * (n + 1)], psst,
                            cwc_bc[:, 512 * n : 512 * (n + 1)], ADD,
                        )
                    nc.scalar.activation(
                        e1[:, jt, :], st_sb, EXP,
                        bias=bias1[:, jt : jt + 1], scale=mq_sb[:, jt : jt + 1],
                        accum_out=z1[:, jt : jt + 1],
                    )

                # ---- E2: natural layout [i part, j free] ----
                e2n = big.tile([128, NT1, L2], BF16, tag="e2n")
                for it in range(NT1):
                    pss = ps512.tile([128, 512], F32, tag="mm512")
                    nc.tensor.matmul(pss, cwt[:, it, :], qt, start=True, stop=True)
                    nc.scalar.activation(
                        e2n[:, it, :], pss, EXP,
                        bias=bias2[:, it : it + 1], scale=mp_sb[:, it : it + 1],
                    )

                # ---- t = s2^T @ C (with Z2 via ones column), rhs_ab = [Q/Z1 | t] ----
                rz1 = work.tile([128, NT2], F32, tag="rz1")
                nc.vector.reciprocal(rz1, z1)
                rhs_ab = big.tile([128, NT2, 256], BF16, tag="rhs_ab")
                for jt in range(NT2):
                    pst = ps256.tile([128, 129], F32, tag="acc")
                    for it in range(NT1):
                        nc.tensor.matmul(
                            pst, e2n[:, it, jt * 128 : (jt + 1) * 128], c1b[:, it, :],
                            start=(it == 0), stop=(it == NT1 - 1),
                        )
                    rz2 = work.tile([128, 1], F32, tag="rz2")
                    nc.vector.reciprocal(rz2, pst[:, 128:129])
                    rz12 = work.tile([128, 1], F32, tag="rz12")
                    nc.vector.tensor_mul(rz12, rz2, rz1[:, jt : jt + 1])
                    nc.vector.tensor_scalar_mul(rhs_ab[:, jt, 128:256], pst[:, 0:128], rz12)
                    nc.vector.tensor_scalar_mul(
                        rhs_ab[:, jt, 0:128], qnb[:, jt, :], rz1[:, jt : jt + 1]
                    )

                # ---- [a | b] = E1^T @ rhs_ab ; assemble output ----
                for it in range(NT1):
                    psab = ps256.tile([128, 256], F32, tag="acc")
                    for jt in range(NT2):
                        nc.tensor.matmul(
                            psab, e1[:, jt, it * 128 : (it + 1) * 128], rhs_ab[:, jt, :],
                            start=(jt == 0), stop=(jt == NT2 - 1),
                        )
                    o_sb = outp.tile([128, 384], F32, tag="o_sb")
                    nc.scalar.copy(o_sb[:, 0:128], psab[:, 0:128])
                    nc.vector.tensor_mul(o_sb[:, 128:256], c1[:, it, :], psab[:, 0:128])
                    nc.vector.tensor_mul(o_sb[:, 256:384], c1[:, it, :], psab[:, 128:256])
                    nc.sync.dma_start(
                        out=out_d[b, it * 128 : (it + 1) * 128, 128:512], in_=o_sb
                    )
                    nc.sync.dma_start(
                        out=out_d[b, it * 128 : (it + 1) * 128, 0:128],
                        in_=c1[:, it, :],
                    )

                if dbg and b == 0:
                    dbg_srcs = {
                        "dbg_e1": e1, "dbg_e2n": e2n, "dbg_z1": z1,
                        "dbg_cwc_row": cwc_row, "dbg_bias2": bias2,
                        "dbg_rhs_ab": rhs_ab, "dbg_ct": ct, "dbg_qt": qt,
                    }
                    for name, src in dbg_srcs.items():
                        dd = nc.dram_tensor(
                            name, list(src.shape), src.dtype, kind="ExternalOutput"
                        ).ap()
                        nc.sync.dma_start(out=dd, in_=src)

    nc.compile()
    return nc


_NC = None


def _get_nc():
    global _NC
    if _NC is None:
        _NC = _build_program()
    return _NC


def _make_in_maps(inputs):
    context, query, w = inputs["context"], inputs["query"], inputs["w"]
    w2 = np.ascontiguousarray(np.asarray(w).reshape(3, D).astype(np.float32))
    mp = np.asarray(inputs["mask_p"]).astype(np.float32)
    mq = np.asarray(inputs["mask_q"]).astype(np.float32)
    in_maps = []
    for c in range(NCORES):
        sl = slice(c * BPC, (c + 1) * BPC)
        in_maps.append(
            {
                "context": np.ascontiguousarray(context[sl]),
                "query": np.ascontiguousarray(query[sl]),
                "w": w2,
                "mask_p": np.ascontiguousarray(mp[sl]),
                "mask_q": np.ascontiguousarray(mq[sl]),
            }
        )
    return in_maps


def kernel(context, query, w, mask_p, mask_q):
    nc = _get_nc()
    in_maps = _make_in_maps(
        {"context": context, "query": query, "w": w, "mask_p": mask_p, "mask_q": mask_q}
    )
    res = bass_utils.run_bass_kernel_spmd(nc, in_maps, core_ids=list(range(NCORES)))
    return np.concatenate([res.results[c]["out"] for c in range(NCORES)], axis=0)


# revision 14
# speedup vs baseline: 2.3813x; 1.2104x over previous
"""ContextQueryAttention (BiDAF-style) Trainium2 kernel, 8-core data-parallel.

Math (per batch):
  s[i,j]  = wq.q_j + wc.c_i + sum_d c_id * wcq_d * q_jd          (L1 x L2)
  s1      = softmax_i(s * mq_j + (1-mq_j)*NEG)                   (softmax over i)
  s2      = softmax_i(s * mp_i + (1-mp_i)*NEG)
  a       = s1 @ Q                 (L1 x D)
  b       = (s1 @ s2^T) @ C  ==  s1 @ (s2^T @ C)   <- reassociated, no L1xL1
  out     = [C, a, C*a, C*b]                                      (L1 x 4D)

Key kernel facts:
 - scores ~ N(0,1): no max-subtraction needed for a stable softmax.
 - qwq_j is constant along the softmax axis (i) in both softmaxes, so it
   cancels in s1 and s2 entirely and is never computed.
 - E1 path, ST layout [j part, i free]: E1 = exp(mq_j*(dot+cwc_i+1000) -
   1000*mq_j); cwc_i+1000 added in f32 via a DMA-broadcast row (bf16 would
   quantize +-2 at magnitude 1000); masked col -> exp(0)=1 -> uniform 1/L1,
   exactly matching the reference. Z1 via ACT accum_out.
 - E2 path, natural layout [i part, j free] from a 2nd score matmul:
   E2 = exp(mp_i*dot + (mp_i*(cwc_i+1000) - 1000)) fully fused in one ACT op
   (per-partition scale+bias); masked entries underflow to exactly 0.
   Z2 via a ones-column appended to C in the t matmul.
 - matmul operands bf16 (fp32 matmul runs as 2 HW passes + slow LDWEIGHTS),
   accumulation f32 in PSUM; exp inputs/outputs of the score pipeline stay
   f32 until the post-exp tensors (values in [e-6, e6], bf16-safe).
"""

import numpy as np

import concourse.bass as bass
import concourse.mybir as mybir
import concourse.tile as tile
from concourse import bacc
from concourse import bass_utils
from concourse.masks import make_identity

F32 = mybir.dt.float32
BF16 = mybir.dt.bfloat16
EXP = mybir.ActivationFunctionType.Exp
IDENT = mybir.ActivationFunctionType.Identity
ADD = mybir.AluOpType.add
MULT = mybir.AluOpType.mult

B, L1, L2, D = 16, 2048, 512, 128
NCORES = 8
BPC = B // NCORES          # batches per core
NT1 = L1 // 128            # 16 i-tiles
NT2 = L2 // 128            # 4  j-tiles
SHIFT = 1000.0             # makes masked E2 entries underflow exp to 0.0


def _build_program(dbg=False):
    nc = bacc.Bacc("TRN2", target_bir_lowering=False, debug=False)

    ctx_d = nc.dram_tensor("context", [BPC, L1, D], F32, kind="ExternalInput").ap()
    qry_d = nc.dram_tensor("query", [BPC, L2, D], F32, kind="ExternalInput").ap()
    w_d = nc.dram_tensor("w", [3, D], F32, kind="ExternalInput").ap()
    mp_d = nc.dram_tensor("mask_p", [BPC, L1], F32, kind="ExternalInput").ap()
    mq_d = nc.dram_tensor("mask_q", [BPC, L2], F32, kind="ExternalInput").ap()
    out_d = nc.dram_tensor("out", [BPC, L1, 4 * D], F32, kind="ExternalOutput").ap()
    dbg_srcs = {}

    with tile.TileContext(nc) as tc:
        with (
            tc.tile_pool(name="const", bufs=1) as const,
            tc.tile_pool(name="big", bufs=2) as big,
            tc.tile_pool(name="work", bufs=2) as work,
            tc.tile_pool(name="outp", bufs=3) as outp,
            tc.tile_pool(name="ps512", bufs=3, space="PSUM") as ps512,
            tc.tile_pool(name="ps256", bufs=3, space="PSUM") as ps256,
            tc.tile_pool(name="psrow", bufs=1, space="PSUM") as psrow,
        ):
            ident_b = const.tile([128, 128], BF16)
            make_identity(nc, ident_b)
            w_sb = const.tile([128, 3], F32)  # cols: wq, wc, wcq
            nc.sync.dma_start(out=w_sb, in_=w_d.rearrange("k d -> d k"))
            w_b = const.tile([128, 3], BF16)
            nc.vector.tensor_copy(w_b, w_sb)
            shift_col = const.tile([128, 1], F32)
            nc.vector.memset(shift_col, SHIFT)

            for b in range(BPC):
                # ---- inputs for this batch ----
                c1 = big.tile([128, NT1, 128], F32, tag="c1")
                nc.sync.dma_start(
                    out=c1, in_=ctx_d[b].rearrange("(t p) d -> p t d", p=128)
                )
                c1b = big.tile([128, NT1, 129], BF16, tag="c1b")  # bf16 C | ones
                for it in range(NT1):
                    nc.any.tensor_copy(c1b[:, it, 0:128], c1[:, it, :])
                nc.vector.memset(c1b[:, :, 128:129], 1.0)
                qn = big.tile([128, NT2, 128], F32, tag="qn")
                nc.sync.dma_start(
                    out=qn, in_=qry_d[b].rearrange("(t p) d -> p t d", p=128)
                )
                qnb = big.tile([128, NT2, 128], BF16, tag="qnb")
                for jt in range(NT2):
                    nc.any.tensor_copy(qnb[:, jt, :], qn[:, jt, :])
                mp_sb = work.tile([128, NT1], F32, tag="mp")
                nc.sync.dma_start(out=mp_sb, in_=mp_d[b].rearrange("(t p) -> p t", p=128))
                mq_sb = work.tile([128, NT2], F32, tag="mq")
                nc.sync.dma_start(out=mq_sb, in_=mq_d[b].rearrange("(t p) -> p t", p=128))

                # ---- transposes: CT [d, i], QT [d, j] (bf16) ----
                ct = big.tile([128, NT1, 128], BF16, tag="ct")
                for it in range(NT1):
                    ps = ps256.tile([128, 128], BF16, tag="acc")
                    nc.tensor.transpose(ps, c1b[:, it, 0:128], ident_b)
                    nc.any.tensor_copy(ct[:, it, :], ps)
                qt = big.tile([128, NT2, 128], BF16, tag="qt")
                for jt in range(NT2):
                    ps = ps256.tile([128, 128], BF16, tag="acc")
                    nc.tensor.transpose(ps, qnb[:, jt, :], ident_b)
                    nc.any.tensor_copy(qt[:, jt, :], ps)

                # ---- wcq-weighted transposes ----
                qtw = big.tile([128, NT2, 128], BF16, tag="qtw")
                nc.vector.tensor_scalar_mul(qtw, qt, w_sb[:, 2:3])
                cwt = big.tile([128, NT1, 128], BF16, tag="cwt")
                nc.vector.tensor_scalar_mul(cwt, ct, w_sb[:, 2:3])

                # ---- cwc row [1, L1] (+SHIFT), f32 broadcast tile ----
                cwc_row = work.tile([1, L1], F32, tag="cwc_row")
                for n in range(L1 // 512):
                    psr = psrow.tile([1, 512], F32, tag="cwcr")
                    nc.tensor.matmul(
                        psr, w_b[:, 1:2], ct[:, 4 * n : 4 * (n + 1), :],
                        start=True, stop=True,
                    )
                    nc.scalar.activation(
                        cwc_row[:, 512 * n : 512 * (n + 1)], psr, IDENT,
                        bias=shift_col[0:1, :],
                    )
                cwc_bc = big.tile([128, L1], F32, tag="cwc_bc")
                nc.gpsimd.partition_broadcast(cwc_bc, cwc_row)

                # cwc in natural (per-partition) form [i_part, it]
                cwc_nat = work.tile([128, NT1], F32, tag="cwc_nat")
                for it in range(NT1):
                    psc = ps256.tile([128, 1], F32, tag="acc")
                    nc.tensor.matmul(psc, ct[:, it, :], w_b[:, 1:2], start=True, stop=True)
                    nc.scalar.copy(cwc_nat[:, it : it + 1], psc)

                # ---- per-partition exp coefficients ----
                bias1 = work.tile([128, NT2], F32, tag="bias1")  # -SHIFT*mq
                nc.vector.tensor_scalar_mul(bias1, mq_sb, -SHIFT)
                bias2 = work.tile([128, NT1], F32, tag="bias2")  # mp*(cwc+SHIFT)-SHIFT
                nc.vector.scalar_tensor_tensor(
                    out=bias2, in0=cwc_nat, scalar=SHIFT, in1=mp_sb,
                    op0=ADD, op1=MULT,
                )
                nc.vector.tensor_scalar_add(bias2, bias2, -SHIFT)

                # ---- E1: ST layout [j part, i free] ----
                e1 = big.tile([128, NT2, L1], BF16, tag="e1")
                z1 = work.tile([128, NT2], F32, tag="z1")
                for jt in range(NT2):
                    st_sb = work.tile([128, L1], F32, tag="st_sb")
                    for n in range(L1 // 512):
                        psst = ps512.tile([128, 512], F32, tag="mm512")
                        nc.tensor.matmul(
                            psst, qtw[:, jt, :], ct[:, 4 * n : 4 * (n + 1), :],
                            start=True, stop=True,
                        )
                        nc.vector.tensor_tensor(
                            st_sb[:, 512 * n : 512 * (n + 1)], psst,
                            cwc_bc[:, 512 * n : 512 * (n + 1)], ADD,
                        )
                    nc.scalar.activation(
                        e1[:, jt, :], st_sb, EXP,
                        bias=bias1[:, jt : jt + 1], scale=mq_sb[:, jt : jt + 1],
                        accum_out=z1[:, jt : jt + 1],
                    )

                # ---- E2: natural layout [i part, j free] ----
                e2n = big.tile([128, NT1, L2], BF16, tag="e2n")
                for it in range(NT1):
                    pss = ps512.tile([128, 512], F32, tag="mm512")
                    nc.tensor.matmul(pss, cwt[:, it, :], qt, start=True, stop=True)
                    nc.scalar.activation(
                        e2n[:, it, :], pss, EXP,
                        bias=bias2[:, it : it + 1], scale=mp_sb[:, it : it + 1],
                    )

                # ---- t = s2^T @ C (with Z2 via ones column), rhs_ab = [Q/Z1 | t] ----
                rz1 = work.tile([128, NT2], F32, tag="rz1")
                nc.vector.reciprocal(rz1, z1)
                rhs_ab = big.tile([128, NT2, 256], BF16, tag="rhs_ab")
                for jt in range(NT2):
                    pst = ps256.tile([128, 129], F32, tag="acc")
                    for it in range(NT1):
                        nc.tensor.matmul(
                            pst, e2n[:, it, jt * 128 : (jt + 1) * 128], c1b[:, it, :],
                            start=(it == 0), stop=(it == NT1 - 1),
                        )
                    rz2 = work.tile([128, 1], F32, tag="rz2")
                    nc.vector.reciprocal(rz2, pst[:, 128:129])
                    rz12 = work.tile([128, 1], F32, tag="rz12")
                    nc.vector.tensor_mul(rz12, rz2, rz1[:, jt : jt + 1])
                    nc.vector.tensor_scalar_mul(rhs_ab[:, jt, 128:256], pst[:, 0:128], rz12)
                    nc.vector.tensor_scalar_mul(
                        rhs_ab[:, jt, 0:128], qnb[:, jt, :], rz1[:, jt : jt + 1]
                    )

                # ---- [a | b] = E1^T @ rhs_ab ; assemble output ----
                for it in range(NT1):
                    psab = ps256.tile([128, 256], F32, tag="acc")
                    for jt in range(NT2):
                        nc.tensor.matmul(
                            psab, e1[:, jt, it * 128 : (it + 1) * 128], rhs_ab[:, jt, :],
                            start=(jt == 0), stop=(jt == NT2 - 1),
                        )
                    o_sb = outp.tile([128, 384], F32, tag="o_sb")
                    nc.any.tensor_copy(o_sb[:, 0:128], psab[:, 0:128])
                    nc.vector.tensor_mul(o_sb[:, 128:256], c1[:, it, :], psab[:, 0:128])
                    nc.vector.tensor_mul(o_sb[:, 256:384], c1[:, it, :], psab[:, 128:256])
                    nc.sync.dma_start(
                        out=out_d[b, it * 128 : (it + 1) * 128, 128:512], in_=o_sb
                    )
                    nc.sync.dma_start(
                        out=out_d[b, it * 128 : (it + 1) * 128, 0:128],
                        in_=c1[:, it, :],
                    )

                if dbg and b == 0:
                    dbg_srcs = {
                        "dbg_e1": e1, "dbg_e2n": e2n, "dbg_z1": z1,
                        "dbg_cwc_row": cwc_row, "dbg_bias2": bias2,
                        "dbg_rhs_ab": rhs_ab, "dbg_ct": ct, "dbg_qt": qt,
                    }
                    for name, src in dbg_srcs.items():
                        dd = nc.dram_tensor(
                            name, list(src.shape), src.dtype, kind="ExternalOutput"
                        ).ap()
                        nc.sync.dma_start(out=dd, in_=src)

    nc.compile()
    return nc


_NC = None


def _get_nc():
    global _NC
    if _NC is None:
        _NC = _build_program()
    return _NC


def _make_in_maps(inputs):
    context, query, w = inputs["context"], inputs["query"], inputs["w"]
    w2 = np.ascontiguousarray(np.asarray(w).reshape(3, D).astype(np.float32))
    mp = np.asarray(inputs["mask_p"]).astype(np.float32)
    mq = np.asarray(inputs["mask_q"]).astype(np.float32)
    in_maps = []
    for c in range(NCORES):
        sl = slice(c * BPC, (c + 1) * BPC)
        in_maps.append(
            {
                "context": np.ascontiguousarray(context[sl]),
                "query": np.ascontiguousarray(query[sl]),
                "w": w2,
                "mask_p": np.ascontiguousarray(mp[sl]),
                "mask_q": np.ascontiguousarray(mq[sl]),
            }
        )
    return in_maps


def kernel(context, query, w, mask_p, mask_q):
    nc = _get_nc()
    in_maps = _make_in_maps(
        {"context": context, "query": query, "w": w, "mask_p": mask_p, "mask_q": mask_q}
    )
    res = bass_utils.run_bass_kernel_spmd(nc, in_maps, core_ids=list(range(NCORES)))
    return np.concatenate([res.results[c]["out"] for c in range(NCORES)], axis=0)
